# revision 11
# baseline (speedup 1.0000x reference)
"""AttnDecoderRNN single-step kernel for 8x TRN2 NeuronCores (Bass/Tile).

Sharding:
  - encoder_outputs: batch-sharded (8 batches/core).
  - out_W: vocab-row-sharded (6400 padded rows/core; V padded 50257 -> 51200).
  - comb_W / GRU weights: output-feature-row-sharded (128 rows/core).
  - attn_W: replicated.
  - embedding: gathered on host (64 rows of the table), passed as cat1T.

Device layout: activations kept transposed [feature(part), batch(free)].
Weights are transposed on-chip via PE-transpose (contraction dim must be on
partitions for both matmul operands).  bf16 is used for the two big
memory-bound matmuls (encoder einsum, out projection); everything else f32.
Collectives: AllGather(attn_applied), AllGather(x), AllGather(new_h),
AllReduce(sum exp logits).
"""

import numpy as np

B, E, H, L, V = 64, 1024, 1024, 512, 50257
NC = 8
BL = B // NC          # 8 local batches per core (encoder shard)
FS = H // NC          # 128 feature rows per core (comb/gru shard)
VS = 6400             # padded vocab rows per core (6400*8 = 51200)
VPAD = VS * NC        # 51200
NCHUNK = (VS + 511) // 512   # 13 chunks; last chunk is 256 wide

_CACHED = {}


def _build():
    import concourse.bass as bass
    import concourse.bacc as bacc
    import concourse.mybir as mybir
    from concourse.tile import TileContext
    from concourse.masks import make_identity

    dt = mybir.dt
    AF = mybir.ActivationFunctionType
    AX = mybir.AxisListType

    nc = bacc.Bacc("TRN2", target_bir_lowering=False, num_devices=NC)
    rg = [list(range(NC))]

    # ---------------- I/O ----------------
    enc_in = nc.dram_tensor("enc_in", [BL, L, H], dt.float32, kind="ExternalInput")
    cat1T_in = nc.dram_tensor("cat1T_in", [E + H, B], dt.float32, kind="ExternalInput")
    cat1Tloc_in = nc.dram_tensor("cat1Tloc_in", [E + H, BL], dt.float32, kind="ExternalInput")
    hidT_in = nc.dram_tensor("hidT_in", [FS, B], dt.float32, kind="ExternalInput")
    attnW_in = nc.dram_tensor("attnW_in", [L, E + H], dt.float32, kind="ExternalInput")
    attnb_in = nc.dram_tensor("attnb_in", [1, L], dt.float32, kind="ExternalInput")
    combW_in = nc.dram_tensor("combW_in", [FS, E + H], dt.float32, kind="ExternalInput")
    wih_in = nc.dram_tensor("wih_in", [3 * FS, E], dt.float32, kind="ExternalInput")
    whh_in = nc.dram_tensor("whh_in", [3 * FS, H], dt.float32, kind="ExternalInput")
    bias_in = nc.dram_tensor("bias_in", [FS, 5], dt.float32, kind="ExternalInput")
    outW_in = nc.dram_tensor("outW_in", [VS, H], dt.float32, kind="ExternalInput")
    outb_in = nc.dram_tensor("outb_in", [1, NCHUNK * 512], dt.float32, kind="ExternalInput")

    logits_out = nc.dram_tensor("logits_out", [B, VS], dt.float32, kind="ExternalOutput")
    nhT_out = nc.dram_tensor("nhT_out", [H, B], dt.float32, kind="ExternalOutput")

    with TileContext(nc) as tc:
        with (
            tc.tile_pool(name="const", bufs=1) as cpool,
            tc.tile_pool(name="big", bufs=1) as big,
            tc.tile_pool(name="gw", bufs=3) as gwp,         # [128,1024] f32 weight chunks
            tc.tile_pool(name="ow", bufs=2) as owp,         # out_W raw bf16 staging
            tc.tile_pool(name="wtc", bufs=8) as wtcp,       # out_W transposed chunk ring
            tc.tile_pool(name="wt", bufs=2) as wtp,         # transposed weight staging
            tc.tile_pool(name="enc", bufs=2) as encp,
            tc.tile_pool(name="act", bufs=1) as actp,       # small activations
            tc.tile_pool(name="sp", bufs=2) as spp,         # tiny rotating tiles
            tc.tile_pool(name="ptr", bufs=3, space="PSUM") as ptr,   # transpose psums
            tc.tile_pool(name="pmm", bufs=4, space="PSUM") as pmm,   # matmul psums
            tc.tile_pool(name="dram", bufs=1, space="DRAM") as dram,
        ):
            # ---------------- constants & small loads ----------------
            ident_f = cpool.tile([128, 128], dt.float32, tag="idf")
            make_identity(nc, ident_f[:])
            ident_b = cpool.tile([128, 128], dt.bfloat16, tag="idb")
            make_identity(nc, ident_b[:])
            ones_f = cpool.tile([1, B], dt.float32, tag="onf")
            nc.gpsimd.memset(ones_f[:], 1.0)
            ones_b = cpool.tile([1, B], dt.bfloat16, tag="onb")
            nc.gpsimd.memset(ones_b[:], 1.0)

            cat1T = cpool.tile([128, 16 * B], dt.float32, tag="cat1")  # (t, b)
            nc.sync.dma_start(cat1T[:], cat1T_in.ap().rearrange("(t p) b -> p t b", p=128))
            cat1Tl = cpool.tile([128, 16 * BL], dt.float32, tag="cat1l")  # (t, bl)
            nc.sync.dma_start(cat1Tl[:], cat1Tloc_in.ap().rearrange("(t p) b -> p t b", p=128))
            hidT = cpool.tile([FS, B], dt.float32, tag="hidT")
            nc.sync.dma_start(hidT[:], hidT_in.ap())
            biases = cpool.tile([FS, 5], dt.float32, tag="bias")
            nc.sync.dma_start(biases[:], bias_in.ap())
            attnb = cpool.tile([1, L], dt.float32, tag="attnb")
            nc.sync.dma_start(attnb[:], attnb_in.ap())

            # persistent big buffers
            logits = big.tile([B, VS], dt.float32, tag="logits")
            nhTb = big.tile([128, 8 * B], dt.bfloat16, tag="nhTb")   # (k, b)
            s_parts = actp.tile([B, 16], dt.float32, tag="sparts")

            # ============ phase 1: attention logits + softmax ================
            # attn_W [512, 2048]; load k-major: group g covers eh cols
            # [g*512,(g+1)*512) for all 512 l rows: tile [128, (c=4, f=512)]
            awr = attnW_in.ap().rearrange("(c p) (g f) -> g p c f", p=128, f=512)
            att_ps = pmm.tile([BL, L], dt.float32, tag="mm")
            for g in range(4):
                awt_a = gwp.tile([128, 1024], dt.float32, tag="gw")
                nc.sync.dma_start(awt_a[:], awr[g][:, 0:2, :])
                awt_b = gwp.tile([128, 1024], dt.float32, tag="gw")
                nc.sync.dma_start(awt_b[:], awr[g][:, 2:4, :])
                for kk in range(4):
                    k = g * 4 + kk
                    awTk = wtp.tile([128, L], dt.float32, tag="awT")
                    for c in range(4):
                        awt = awt_a if c < 2 else awt_b
                        pt = ptr.tile([128, 128], dt.float32, tag="tr")
                        nc.tensor.transpose(pt[:], awt[:, (c % 2) * 512 + kk * 128: (c % 2) * 512 + (kk + 1) * 128], ident_f[:])
                        nc.vector.tensor_copy(awTk[:, c * 128:(c + 1) * 128], pt[:])
                    nc.tensor.matmul(att_ps[:], cat1Tl[:, k * BL:(k + 1) * BL], awTk[:],
                                     start=(k == 0), stop=False)
            nc.tensor.matmul(att_ps[:], ones_f[0:1, 0:BL], attnb[:], start=False, stop=True)

            # softmax over free dim (L)
            m = actp.tile([BL, 1], dt.float32, tag="m")
            nc.vector.reduce_max(out=m[:], in_=att_ps[:], axis=AX.X)
            negm = actp.tile([BL, 1], dt.float32, tag="negm")
            nc.vector.tensor_scalar_mul(negm[:], m[:], -1.0)
            ex = actp.tile([BL, L], dt.float32, tag="ex")
            s = actp.tile([BL, 1], dt.float32, tag="s")
            nc.scalar.activation(ex[:], att_ps[:], AF.Exp, bias=negm[:, 0:1], scale=1.0,
                                 accum_out=s[:, 0:1])
            rs = actp.tile([BL, 1], dt.float32, tag="rs")
            nc.vector.reciprocal(rs[:], s[:])
            nc.vector.tensor_scalar_mul(ex[:], ex[:], rs[:, 0:1])
            wsmb = actp.tile([BL, L], dt.bfloat16, tag="wsmb")
            nc.scalar.copy(wsmb[:], ex[:])
            # transpose attn weights -> wT [l(part), (c, b)]
            wT = actp.tile([128, 4 * BL], dt.bfloat16, tag="wT")
            for c in range(4):
                pt = ptr.tile([128, 128], dt.bfloat16, tag="tr")
                nc.tensor.transpose(pt[:, 0:BL], wsmb[:, c * 128:(c + 1) * 128], ident_b[0:BL, 0:BL])
                nc.vector.tensor_copy(wT[:, c * BL:(c + 1) * BL], pt[:, 0:BL])

            # ============ phase 2: einsum bl,bld->bd (local batches) =========
            encr = enc_in.ap().rearrange("j (c p) h -> j p c h", p=128)
            ag_app_i = dram.tile([BL, H], dt.float32, tag="agai")
            ag_app_o = dram.tile([B, H], dt.float32, tag="agao")
            for j in range(BL):
                et = encp.tile([128, 4 * H], dt.bfloat16, tag="enc")
                nc.gpsimd.dma_start(et[:], encr[j])  # cast f32->bf16
                app_lo = pmm.tile([1, 512], dt.float32, tag="mm")
                app_hi = pmm.tile([1, 512], dt.float32, tag="mm")
                for c in range(4):
                    wcol = wT[:, c * BL + j: c * BL + j + 1]
                    nc.tensor.matmul(app_lo[:], wcol, et[:, c * H: c * H + 512],
                                     start=(c == 0), stop=(c == 3))
                    nc.tensor.matmul(app_hi[:], wcol, et[:, c * H + 512: (c + 1) * H],
                                     start=(c == 0), stop=(c == 3))
                appj = spp.tile([1, H], dt.float32, tag="appj")
                nc.scalar.copy(appj[0:1, 0:512], app_lo[:])
                nc.scalar.copy(appj[0:1, 512:H], app_hi[:])
                nc.sync.dma_start(ag_app_i[j:j + 1, :], appj[0:1, :])

            # AllGather attn_applied [BL, H] -> [B, H]
            nc.gpsimd.collective_compute("AllGather", mybir.AluOpType.bypass,
                                         replica_groups=rg,
                                         ins=[ag_app_i.opt()], outs=[ag_app_o.opt()])
            appf = actp.tile([B, H], dt.float32, tag="appf")
            nc.sync.dma_start(appf[:], ag_app_o[:])
            # transpose to cat2T upper half [h(part), (k, b)]
            appT = actp.tile([128, 8 * B], dt.float32, tag="appT")
            for k in range(8):
                pt = ptr.tile([128, 128], dt.float32, tag="tr")
                nc.tensor.transpose(pt[:, 0:B], appf[:, k * 128:(k + 1) * 128], ident_f[0:B, 0:B])
                nc.vector.tensor_copy(appT[:, k * B:(k + 1) * B], pt[:, 0:B])

            # ============ phase 3: comb + relu -> x shard =====================
            x_ps = pmm.tile([FS, B], dt.float32, tag="mm")
            for half in range(2):
                cwt = gwp.tile([128, 1024], dt.float32, tag="gw")
                nc.sync.dma_start(cwt[:], combW_in.ap()[:, half * 1024:(half + 1) * 1024])
                for kk in range(8):
                    k = half * 8 + kk
                    pt = ptr.tile([128, 128], dt.float32, tag="tr")
                    nc.tensor.transpose(pt[:], cwt[:, kk * 128:(kk + 1) * 128], ident_f[:])
                    cwTk = wtp.tile([128, 128], dt.float32, tag="swT")
                    nc.vector.tensor_copy(cwTk[:], pt[:])
                    rhs = cat1T[:, k * B:(k + 1) * B] if k < 8 else appT[:, (k - 8) * B:(k - 7) * B]
                    nc.tensor.matmul(x_ps[:], cwTk[:], rhs, start=(k == 0), stop=(k == 15))
            xb = actp.tile([FS, B], dt.float32, tag="xb")
            nc.scalar.activation(xb[:], x_ps[:], AF.Relu, bias=biases[:, 0:1], scale=1.0)

            # AllGather x [FS, B] -> xT full [H, B]
            ag_x_i = dram.tile([FS, B], dt.float32, tag="agxi")
            ag_x_o = dram.tile([H, B], dt.float32, tag="agxo")
            nc.sync.dma_start(ag_x_i[:], xb[:])
            nc.gpsimd.collective_compute("AllGather", mybir.AluOpType.bypass,
                                         replica_groups=rg,
                                         ins=[ag_x_i.opt()], outs=[ag_x_o.opt()])
            xT = actp.tile([128, 8 * B], dt.float32, tag="xT")
            nc.sync.dma_start(xT[:], ag_x_o[:].rearrange("(k p) b -> p k b", p=128))

            # ============ phase 4: GRU ========================================
            wihr = wih_in.ap().rearrange("(g p) f -> p g f", p=128)
            whhr = whh_in.ap().rearrange("(g p) f -> p g f", p=128)

            p_r = pmm.tile([FS, B], dt.float32, tag="mm")
            p_z = pmm.tile([FS, B], dt.float32, tag="mm")
            p_i = pmm.tile([FS, B], dt.float32, tag="mm")
            p_h = pmm.tile([FS, B], dt.float32, tag="mm")

            def gate_mms(ps, wsrc_ap, g, rhs_fn, start, stop):
                wt_g = gwp.tile([128, 1024], dt.float32, tag="gw")
                nc.sync.dma_start(wt_g[:], wsrc_ap[:, g:g + 1, :])
                for k in range(8):
                    pt = ptr.tile([128, 128], dt.float32, tag="tr")
                    nc.tensor.transpose(pt[:], wt_g[:, k * 128:(k + 1) * 128], ident_f[:])
                    wTk = wtp.tile([128, 128], dt.float32, tag="swT")
                    nc.vector.tensor_copy(wTk[:], pt[:])
                    nc.tensor.matmul(ps[:], wTk[:], rhs_fn(k),
                                     start=(start and k == 0), stop=(stop and k == 7))

            xT_rhs = lambda k: xT[:, k * B:(k + 1) * B]
            hT_rhs = lambda k: cat1T[:, (8 + k) * B:(9 + k) * B]
            gate_mms(p_r, wihr, 0, xT_rhs, True, False)
            gate_mms(p_r, whhr, 0, hT_rhs, False, True)
            gate_mms(p_z, wihr, 1, xT_rhs, True, False)
            gate_mms(p_z, whhr, 1, hT_rhs, False, True)
            gate_mms(p_i, wihr, 2, xT_rhs, True, True)
            gate_mms(p_h, whhr, 2, hT_rhs, True, True)

            r_t = actp.tile([FS, B], dt.float32, tag="r")
            z_t = actp.tile([FS, B], dt.float32, tag="z")
            i_t = actp.tile([FS, B], dt.float32, tag="i")
            h_t = actp.tile([FS, B], dt.float32, tag="h")
            nc.scalar.activation(r_t[:], p_r[:], AF.Sigmoid, bias=biases[:, 1:2], scale=1.0)
            nc.scalar.activation(z_t[:], p_z[:], AF.Sigmoid, bias=biases[:, 2:3], scale=1.0)
            nc.scalar.activation(i_t[:], p_i[:], AF.Identity, bias=biases[:, 3:4], scale=1.0)
            nc.scalar.activation(h_t[:], p_h[:], AF.Identity, bias=biases[:, 4:5], scale=1.0)
            t1 = actp.tile([FS, B], dt.float32, tag="t1")
            nc.vector.tensor_mul(t1[:], r_t[:], h_t[:])
            nc.vector.tensor_add(t1[:], t1[:], i_t[:])
            n_t = actp.tile([FS, B], dt.float32, tag="n")
            nc.scalar.activation(n_t[:], t1[:], AF.Tanh, bias=0.0, scale=1.0)
            d_t = actp.tile([FS, B], dt.float32, tag="d")
            nc.vector.tensor_sub(d_t[:], hidT[:], n_t[:])
            nc.vector.tensor_mul(d_t[:], z_t[:], d_t[:])
            nh_t = actp.tile([FS, B], dt.float32, tag="nh")
            nc.vector.tensor_add(nh_t[:], n_t[:], d_t[:])

            # AllGather new_hidden [FS, B] -> [H, B]
            ag_nh_i = dram.tile([FS, B], dt.float32, tag="agni")
            ag_nh_o = dram.tile([H, B], dt.float32, tag="agno")
            nc.sync.dma_start(ag_nh_i[:], nh_t[:])
            nc.gpsimd.collective_compute("AllGather", mybir.AluOpType.bypass,
                                         replica_groups=rg,
                                         ins=[ag_nh_i.opt()], outs=[ag_nh_o.opt()])
            nhTf = actp.tile([128, 8 * B], dt.float32, tag="nhTf")
            nc.sync.dma_start(nhTf[:], ag_nh_o[:].rearrange("(k p) b -> p k b", p=128))
            nc.scalar.copy(nhTb[:], nhTf[:])  # cast to bf16 for the big matmul
            # write new_hidden output (transposed; host untransposes)
            nc.sync.dma_start(nhT_out.ap().rearrange("(k p) b -> p k b", p=128), nhTf[:])

            # ============ out_W load + transpose stream ======================
            # traced late => lowest DMA/PE priority; runs in idle gaps from t=0
            owr = outW_in.ap().rearrange("(vb r p) h -> vb p r h", p=128, r=2)
            wt_chunks = {}
            for vb in range(VS // 256):
                owt = owp.tile([128, 2048], dt.bfloat16, tag="ow")
                nc.gpsimd.dma_start(owt[:], owr[vb])  # cast f32->bf16
                for r in range(2):
                    vblock = vb * 2 + r
                    ch = (vblock * 128) // 512
                    if ch not in wt_chunks:
                        wtc_t = wtcp.tile([128, 8 * 512], dt.bfloat16, tag="wtc", name=f"wtc{ch}")
                        wt_chunks[ch] = wtc_t
                    off = (vblock * 128) % 512
                    for k in range(8):
                        pt = ptr.tile([128, 128], dt.bfloat16, tag="tr")
                        nc.tensor.transpose(pt[:], owt[:, r * 1024 + k * 128: r * 1024 + (k + 1) * 128], ident_b[:])
                        dst = wt_chunks[ch][:, k * 512 + off: k * 512 + off + 128]
                        if (vblock * 8 + k) % 2 == 0:
                            nc.vector.tensor_copy(dst, pt[:])
                        else:
                            nc.scalar.copy(dst, pt[:])

            # ============ phase 5: out projection + log-softmax ==============
            etmp = actp.tile([B, 512], dt.float32, tag="etmp")
            for c in range(NCHUNK):
                n_c = min(512, VS - c * 512)
                obc = spp.tile([1, 512], dt.bfloat16, tag="obc")
                nc.gpsimd.dma_start(obc[0:1, 0:n_c], outb_in.ap()[0:1, c * 512: c * 512 + n_c])
                po = pmm.tile([B, 512], dt.float32, tag="mm")
                for k in range(8):
                    nc.tensor.matmul(po[:, 0:n_c], nhTb[:, k * B:(k + 1) * B],
                                     wt_chunks[c][:, k * 512: k * 512 + n_c],
                                     start=(k == 0), stop=False)
                nc.tensor.matmul(po[:, 0:n_c], ones_b[:], obc[0:1, 0:n_c],
                                 start=False, stop=True)
                nc.vector.tensor_copy(logits[:, c * 512: c * 512 + n_c], po[:, 0:n_c])
                nc.scalar.activation(etmp[:, 0:n_c], po[:, 0:n_c], AF.Exp,
                                     bias=0.0, scale=1.0, accum_out=s_parts[:, c:c + 1])

            s_loc = actp.tile([B, 1], dt.float32, tag="sloc")
            nc.vector.reduce_sum(out=s_loc[:], in_=s_parts[:, 0:NCHUNK], axis=AX.X)
            ag_s_i = dram.tile([B, 1], dt.float32, tag="agsi")
            ag_s_o = dram.tile([B, 1], dt.float32, tag="agso")
            nc.sync.dma_start(ag_s_i[:], s_loc[:])
            nc.gpsimd.collective_compute("AllReduce", mybir.AluOpType.add,
                                         replica_groups=rg,
                                         ins=[ag_s_i.opt()], outs=[ag_s_o.opt()])
            s_tot = actp.tile([B, 1], dt.float32, tag="stot")
            nc.sync.dma_start(s_tot[:], ag_s_o[:])
            logZ = actp.tile([B, 1], dt.float32, tag="logZ")
            nc.scalar.activation(logZ[:], s_tot[:], AF.Ln, bias=0.0, scale=1.0)
            nc.vector.tensor_scalar_sub(logits[:], logits[:], logZ[:, 0:1])
            nc.sync.dma_start(logits_out.ap(), logits[:])

    if not nc.is_finalized():
        nc.finalize()
    return nc


def _get_nc():
    if "nc" not in _CACHED:
        _CACHED["nc"] = _build()
    return _CACHED["nc"]


def _prep_inputs(input_ids, hidden, encoder_outputs, emb_W, attn_W, attn_b,
                 comb_W, comb_b, gru_Wih, gru_Whh, gru_bih, gru_bhh, out_W, out_b):
    f32 = np.float32
    ids = np.asarray(input_ids).astype(np.int64)
    emb = np.asarray(emb_W)[ids].astype(f32)              # [B, E]
    hid = np.asarray(hidden).astype(f32)                  # [B, H]
    cat1T = np.ascontiguousarray(
        np.concatenate([emb.T, hid.T], axis=0), dtype=f32)  # [E+H, B]
    hidT = np.ascontiguousarray(hid.T, dtype=f32)         # [H, B]
    enc = np.asarray(encoder_outputs, dtype=f32)

    attn_Wc = np.ascontiguousarray(np.asarray(attn_W), dtype=f32)
    attn_b2 = np.ascontiguousarray(np.asarray(attn_b).reshape(1, L), dtype=f32)

    oW = np.asarray(out_W, dtype=f32)
    oWp = np.zeros((VPAD, H), dtype=f32)
    oWp[:V] = oW
    ob = np.full((VPAD,), -1e30, dtype=f32)
    ob[:V] = np.asarray(out_b, dtype=f32)

    gWih = np.asarray(gru_Wih, dtype=f32)
    gWhh = np.asarray(gru_Whh, dtype=f32)
    b_r = (np.asarray(gru_bih)[0:H] + np.asarray(gru_bhh)[0:H]).astype(f32)
    b_z = (np.asarray(gru_bih)[H:2 * H] + np.asarray(gru_bhh)[H:2 * H]).astype(f32)
    b_in = np.asarray(gru_bih)[2 * H:3 * H].astype(f32)
    b_hn = np.asarray(gru_bhh)[2 * H:3 * H].astype(f32)
    cb = np.asarray(comb_b, dtype=f32)

    in_maps = []
    for c in range(NC):
        fs = slice(c * FS, (c + 1) * FS)
        bias5 = np.stack([cb[fs], b_r[fs], b_z[fs], b_in[fs], b_hn[fs]], axis=1)
        obp = np.concatenate([ob[c * VS:(c + 1) * VS],
                              np.zeros(NCHUNK * 512 - VS, dtype=f32)])
        wih_sh = np.concatenate(
            [gWih[g * H + c * FS: g * H + (c + 1) * FS] for g in range(3)], axis=0)
        whh_sh = np.concatenate(
            [gWhh[g * H + c * FS: g * H + (c + 1) * FS] for g in range(3)], axis=0)
        in_maps.append({
            "enc_in": np.ascontiguousarray(enc[c * BL:(c + 1) * BL]),
            "cat1T_in": cat1T,
            "cat1Tloc_in": np.ascontiguousarray(cat1T[:, c * BL:(c + 1) * BL]),
            "hidT_in": np.ascontiguousarray(hidT[c * FS:(c + 1) * FS]),
            "attnW_in": attn_Wc,
            "attnb_in": attn_b2,
            "combW_in": np.ascontiguousarray(np.asarray(comb_W, dtype=f32)[fs]),
            "wih_in": np.ascontiguousarray(wih_sh),
            "whh_in": np.ascontiguousarray(whh_sh),
            "bias_in": np.ascontiguousarray(bias5, dtype=f32),
            "outW_in": np.ascontiguousarray(oWp[c * VS:(c + 1) * VS]),
            "outb_in": np.ascontiguousarray(obp.reshape(1, NCHUNK * 512)),
        })
    return in_maps


def _assemble(results):
    logits = np.concatenate([r["logits_out"] for r in results], axis=1)[:, :V]
    new_hidden = np.ascontiguousarray(results[0]["nhT_out"].T)
    return (np.ascontiguousarray(logits, dtype=np.float32),
            new_hidden.astype(np.float32))


def _run(inputs, **kw):
    from concourse.bass_utils import run_bass_kernel_spmd
    nc = _get_nc()
    in_maps = _prep_inputs(**inputs)
    res = run_bass_kernel_spmd(nc, in_maps, core_ids=list(range(NC)), **kw)
    return _assemble(res.results), res


def kernel(**inputs):
    outs, _ = _run(inputs)
    return outs
